# revision 3
# baseline (speedup 1.0000x reference)
"""Trainium2 Bass kernel for nn_DecoderMixer (L=13, B=4, T=1024, C=1024, H=16).

Sharding: data-parallel over the fused B*T axis — 8 cores x 512 rows.
Each row's 13-token attention is independent; weights replicated.

Device-side algorithm (per core, per 128-row chunk):
  - RoPE is folded into the weights HOST-side (RoPE is a linear map on the
    head dim): Wk_l = R_l @ Wk for l = 0..12 (streamed per l), and
    Wq' = (R_12 @ Wq) / sqrt(D) (only the last query position is ever used,
    since the module returns out[:, -1, :]).
  - q = x_12 @ Wq'.T  (PE, fp32r)
  - online attention over l: K_l/V_l projected into PSUM (PE, fp32r),
    scores = reduce_d(q * K_l) (DVE), e = exp(scores) (ACT),
    den += e, num += e * V_l (DVE). No max-subtraction needed: scores are
    ~N(0,1) with |s| < ~6.
  - att = num / den, PE-transpose, out = att @ Wo.T (PE, fp32r).

fp32r matmuls: measured absmax-relative error ~1e-4 per 1024-contraction
matmul at bf16 speed (1 cycle/row for N>=256).
"""

import numpy as np

import concourse.bass as bass
import concourse.tile as tile
from concourse import bacc, mybir
from concourse.bass_utils import run_bass_kernel_spmd

L, B, T, C = 13, 4, 1024, 1024
H, D = 16, 64
N_CORES = 8
NPC = (B * T) // N_CORES   # 512 rows per core
CHUNK = 128
NCHUNK = NPC // CHUNK      # 4
CI = C // 128              # 8 contraction tiles
ROPE_BASE = 10000.0

F32 = mybir.dt.float32
F32R = mybir.dt.float32r

_CACHED_NC = None


def _emit(tc, aps):
    nc = tc.nc
    xt, xq, wkt, wvt, wqt, wot, ident, out = (
        aps["xt"], aps["xq"], aps["wkt"], aps["wvt"], aps["wqt"], aps["wot"],
        aps["ident"], aps["out"],
    )

    with (
        tc.tile_pool(name="wk", bufs=2) as wk_pool,
        tc.tile_pool(name="x", bufs=2) as x_pool,
        tc.tile_pool(name="res", bufs=1) as res_pool,
        tc.tile_pool(name="small", bufs=4) as small_pool,
        tc.tile_pool(name="p", bufs=2) as p_pool,
        tc.tile_pool(name="o", bufs=1) as o_pool,
        tc.tile_pool(name="ps", bufs=4, space="PSUM") as ps_pool,
    ):
        # ---- resident tensors ----
        wv_sb = res_pool.tile([128, CI, C], F32R, tag="wv")
        for g in range(CI):
            nc.sync.dma_start(wv_sb[:, g, :], wvt[g * 128:(g + 1) * 128, :])
        id_sb = res_pool.tile([128, 128], F32, tag="id")
        nc.sync.dma_start(id_sb[:], ident[:])

        q_sb = res_pool.tile([128, NCHUNK, C], F32, tag="q")
        num_sb = res_pool.tile([128, NCHUNK, H, D], F32, tag="num")
        den_sb = res_pool.tile([128, NCHUNK, H], F32, tag="den")
        nc.gpsimd.memset(num_sb[:], 0.0)
        nc.gpsimd.memset(den_sb[:], 0.0)

        # ---- Q projection (last position only, roped+scaled weights) ----
        wq_sb = wk_pool.tile([128, CI, C], F32R, tag="w")
        for g in range(CI):
            nc.sync.dma_start(wq_sb[:, g, :], wqt[g * 128:(g + 1) * 128, :])
        xq_sb = x_pool.tile([128, CI, NPC], F32R, tag="x")
        for g in range(CI):
            nc.sync.dma_start(xq_sb[:, g, :], xq[g * 128:(g + 1) * 128, :])

        for ch in range(NCHUNK):
            q_ps = ps_pool.tile([128, C], F32, tag="kv")
            cs = slice(ch * CHUNK, (ch + 1) * CHUNK)
            for half in range(2):
                hs = slice(half * 512, (half + 1) * 512)
                for g in range(CI):
                    nc.tensor.matmul(
                        q_ps[:, hs], xq_sb[:, g, cs], wq_sb[:, g, hs],
                        start=(g == 0), stop=(g == CI - 1),
                    )
            nc.scalar.copy(q_sb[:, ch, :], q_ps[:])

        # ---- online attention over l ----
        for l in range(L):
            wk_sb = wk_pool.tile([128, CI, C], F32R, tag="w")
            for g in range(CI):
                nc.sync.dma_start(wk_sb[:, g, :], wkt[l, g * 128:(g + 1) * 128, :])
            x_sb = x_pool.tile([128, CI, NPC], F32R, tag="x")
            for g in range(CI):
                nc.sync.dma_start(x_sb[:, g, :], xt[l, g * 128:(g + 1) * 128, :])

            for ch in range(NCHUNK):
                cs = slice(ch * CHUNK, (ch + 1) * CHUNK)
                k_ps = ps_pool.tile([128, C], F32, tag="kv")
                v_ps = ps_pool.tile([128, C], F32, tag="kv")
                for half in range(2):
                    hs = slice(half * 512, (half + 1) * 512)
                    for g in range(CI):
                        nc.tensor.matmul(
                            k_ps[:, hs], x_sb[:, g, cs], wk_sb[:, g, hs],
                            start=(g == 0), stop=(g == CI - 1),
                        )
                for half in range(2):
                    hs = slice(half * 512, (half + 1) * 512)
                    for g in range(CI):
                        nc.tensor.matmul(
                            v_ps[:, hs], x_sb[:, g, cs], wv_sb[:, g, hs],
                            start=(g == 0), stop=(g == CI - 1),
                        )

                # scores: s[n, h] = sum_d q[n, h, d] * k[n, h, d]
                p_sb = p_pool.tile([128, H, D], F32, tag="p")
                nc.vector.tensor_mul(
                    p_sb[:],
                    q_sb[:, ch, :].rearrange("p (h d) -> p h d", d=D),
                    k_ps[:].rearrange("p (h d) -> p h d", d=D),
                )
                s_sb = small_pool.tile([128, H], F32, tag="s")
                nc.vector.tensor_reduce(
                    s_sb[:], p_sb[:], axis=mybir.AxisListType.X,
                    op=mybir.AluOpType.add,
                )
                e_sb = small_pool.tile([128, H], F32, tag="e")
                nc.scalar.activation(
                    e_sb[:], s_sb[:], mybir.ActivationFunctionType.Exp,
                )
                nc.vector.tensor_add(
                    den_sb[:, ch, :], den_sb[:, ch, :], e_sb[:]
                )
                # num += e * v
                m_sb = p_pool.tile([128, H, D], F32, tag="m")
                nc.vector.tensor_mul(
                    m_sb[:],
                    v_ps[:].rearrange("p (h d) -> p h d", d=D),
                    e_sb[:].unsqueeze(2).broadcast_to((128, H, D)),
                )
                nc.vector.tensor_add(num_sb[:, ch], num_sb[:, ch], m_sb[:])

        # ---- normalize + output projection ----
        wo_sb = wk_pool.tile([128, CI, C], F32R, tag="w")
        for g in range(CI):
            nc.sync.dma_start(wo_sb[:, g, :], wot[g * 128:(g + 1) * 128, :])

        for ch in range(NCHUNK):
            rden = small_pool.tile([128, H], F32, tag="rd")
            nc.vector.reciprocal(rden[:], den_sb[:, ch, :])
            att_sb = o_pool.tile([128, H, D], F32, tag="att")
            nc.vector.tensor_mul(
                att_sb[:], num_sb[:, ch],
                rden[:].unsqueeze(2).broadcast_to((128, H, D)),
            )
            att2 = att_sb[:].rearrange("p h d -> p (h d)")
            t_ps = ps_pool.tile([128, C], F32, tag="kv")
            for g in range(CI):
                nc.tensor.transpose(
                    t_ps[:, g * 128:(g + 1) * 128],
                    att2[:, g * 128:(g + 1) * 128],
                    id_sb[:],
                )
            attT = o_pool.tile([128, CI, 128], F32R, tag="attT")
            nc.vector.tensor_copy(
                attT[:].rearrange("p g n -> p (g n)"), t_ps[:]
            )
            o_ps = ps_pool.tile([128, C], F32, tag="kv")
            for half in range(2):
                hs = slice(half * 512, (half + 1) * 512)
                for g in range(CI):
                    nc.tensor.matmul(
                        o_ps[:, hs], attT[:, g, :], wo_sb[:, g, hs],
                        start=(g == 0), stop=(g == CI - 1),
                    )
            out_sb = o_pool.tile([128, C], F32, tag="out")
            nc.scalar.copy(out_sb[:], o_ps[:])
            nc.sync.dma_start(out[ch * CHUNK:(ch + 1) * CHUNK, :], out_sb[:])


def _build_bass():
    nc = bacc.Bacc("TRN2", target_bir_lowering=False, debug=False,
                   num_devices=N_CORES)
    aps = {
        "xt": nc.dram_tensor("xt", (L, C, NPC), F32R, kind="ExternalInput").ap(),
        "xq": nc.dram_tensor("xq", (C, NPC), F32R, kind="ExternalInput").ap(),
        "wkt": nc.dram_tensor("wkt", (L, C, C), F32R, kind="ExternalInput").ap(),
        "wvt": nc.dram_tensor("wvt", (C, C), F32R, kind="ExternalInput").ap(),
        "wqt": nc.dram_tensor("wqt", (C, C), F32R, kind="ExternalInput").ap(),
        "wot": nc.dram_tensor("wot", (C, C), F32R, kind="ExternalInput").ap(),
        "ident": nc.dram_tensor("ident", (128, 128), F32, kind="ExternalInput").ap(),
        "out": nc.dram_tensor("out", (NPC, C), F32, kind="ExternalOutput").ap(),
    }
    with tile.TileContext(nc) as tc:
        _emit(tc, aps)
    nc.compile()
    return nc


def _rope_tables():
    inv_freq = 1.0 / (ROPE_BASE ** (np.arange(0, D, 2, dtype=np.float32) / D))
    freqs = np.arange(L, dtype=np.float32)[:, None] * inv_freq[None, :]
    emb = np.concatenate([freqs, freqs], axis=-1)          # (L, D)
    return np.cos(emb).astype(np.float32), np.sin(emb).astype(np.float32)


def _rope_weight(w, cos_l, sin_l):
    """R_l @ W for a (C, C) projection weight (rows indexed by h*D+d)."""
    w3 = w.reshape(H, D, C)
    rot = np.concatenate([-w3[:, D // 2:, :], w3[:, :D // 2, :]], axis=1)
    return (cos_l[None, :, None] * w3 + sin_l[None, :, None] * rot).reshape(C, C)


def _host_prep(layer_outputs, Wq, Wk, Wv, Wo):
    cos, sin = _rope_tables()
    wkt = np.empty((L, C, C), dtype=np.float32)
    for l in range(L):
        wkt[l] = np.ascontiguousarray(_rope_weight(Wk, cos[l], sin[l]).T)
    wq12 = _rope_weight(Wq, cos[L - 1], sin[L - 1]) / np.float32(np.sqrt(D))
    shared = {
        "wkt": wkt,
        "wvt": np.ascontiguousarray(Wv.T),
        "wqt": np.ascontiguousarray(wq12.T.astype(np.float32)),
        "wot": np.ascontiguousarray(Wo.T),
        "ident": np.eye(128, dtype=np.float32),
    }
    in_maps = []
    for c in range(N_CORES):
        b = c // (T // NPC) if NPC <= T else c
        # rows n = b*T + t, core c covers n in [c*NPC, (c+1)*NPC)
        n0 = c * NPC
        b = n0 // T
        t0 = n0 % T
        sl = layer_outputs[:, b, t0:t0 + NPC, :]          # (L, NPC, C)
        xt = np.ascontiguousarray(sl.transpose(0, 2, 1))  # (L, C, NPC)
        in_maps.append({
            "xt": xt,
            "xq": np.ascontiguousarray(xt[L - 1]),
            **shared,
        })
    return in_maps


def _get_nc():
    global _CACHED_NC
    if _CACHED_NC is None:
        _CACHED_NC = _build_bass()
    return _CACHED_NC


def kernel(layer_outputs, Wq, Wk, Wv, Wo):
    layer_outputs = np.asarray(layer_outputs, dtype=np.float32)
    Wq = np.asarray(Wq, dtype=np.float32)
    Wk = np.asarray(Wk, dtype=np.float32)
    Wv = np.asarray(Wv, dtype=np.float32)
    Wo = np.asarray(Wo, dtype=np.float32)

    nc = _get_nc()
    in_maps = _host_prep(layer_outputs, Wq, Wk, Wv, Wo)
    res = run_bass_kernel_spmd(nc, in_maps, core_ids=list(range(N_CORES)))
    full = np.concatenate([r["out"] for r in res.results], axis=0)  # (B*T, C)
    return full.reshape(B, T, C)


if __name__ == "__main__":
    nc = _build_bass()
    print("build OK:",
          sum(len(f.blocks[0].instructions) if f.blocks else 0
              for f in nc.m.functions) if hasattr(nc.m.functions[0], 'blocks')
          else "n/a")


# revision 6
# speedup vs baseline: 372.7599x; 372.7599x over previous
"""Trainium2 Bass kernel for nn_DecoderMixer (L=13, B=4, T=1024, C=1024, H=16).

Sharding: data-parallel over the fused B*T axis — 8 cores x 512 rows.
Each row's 13-token attention is independent; weights replicated.

Device-side algorithm (per core, per 128-row chunk):
  - RoPE is folded into the weights HOST-side (RoPE is a linear map on the
    head dim): Wk_l = R_l @ Wk for l = 0..12 (streamed per l), and
    Wq' = (R_12 @ Wq) / sqrt(D) (only the last query position is ever used,
    since the module returns out[:, -1, :]).
  - q = x_12 @ Wq'.T  (PE, fp32r)
  - online attention over l: K_l/V_l projected into PSUM (PE, fp32r),
    scores = reduce_d(q * K_l) (DVE), e = exp(scores) (ACT),
    den += e, num += e * V_l (DVE). No max-subtraction needed: scores are
    ~N(0,1) with |s| < ~6.
  - att = num / den, PE-transpose, out = att @ Wo.T (PE, fp32r).

fp32r matmuls: measured absmax-relative error ~1e-4 per 1024-contraction
matmul at bf16 speed (1 cycle/row for N>=256).
"""

import os

import numpy as np

import concourse.bass as bass
import concourse.tile as tile
from concourse import bacc, mybir
from concourse.bass_utils import run_bass_kernel_spmd

L, B, T, C = 13, 4, 1024, 1024
H, D = 16, 64
N_CORES = 8
NPC = (B * T) // N_CORES   # 512 rows per core
CHUNK = 128
NCHUNK = NPC // CHUNK      # 4
CI = C // 128              # 8 contraction tiles
ROPE_BASE = 10000.0

F32 = mybir.dt.float32
F32R = mybir.dt.float32r

_CACHED_NC = None


def _emit(tc, aps):
    nc = tc.nc
    xt, xq, wkt, wvt, wqt, wot, ident, out = (
        aps["xt"], aps["xq"], aps["wkt"], aps["wvt"], aps["wqt"], aps["wot"],
        aps["ident"], aps["out"],
    )

    with (
        tc.tile_pool(name="wk", bufs=2) as wk_pool,
        tc.tile_pool(name="x", bufs=2) as x_pool,
        tc.tile_pool(name="res", bufs=1) as res_pool,
        tc.tile_pool(name="small", bufs=4) as small_pool,
        tc.tile_pool(name="p", bufs=2) as p_pool,
        tc.tile_pool(name="o", bufs=1) as o_pool,
        tc.tile_pool(name="ps", bufs=4, space="PSUM") as ps_pool,
    ):
        # ---- resident tensors ----
        wv_sb = res_pool.tile([128, CI, C], F32R, tag="wv")
        for g in range(CI):
            nc.sync.dma_start(wv_sb[:, g, :], wvt[g * 128:(g + 1) * 128, :])
        id_sb = res_pool.tile([128, 128], F32, tag="id")
        nc.sync.dma_start(id_sb[:], ident[:])

        q_sb = res_pool.tile([128, NCHUNK, C], F32, tag="q")
        num_sb = res_pool.tile([128, NCHUNK, H, D], F32, tag="num")
        e_all = res_pool.tile([128, NCHUNK, L, H], F32, tag="e_all")
        nc.gpsimd.memset(num_sb[:], 0.0)

        # ---- Q projection (last position only, roped+scaled weights) ----
        wq_sb = wk_pool.tile([128, CI, C], F32R, tag="w")
        for g in range(CI):
            nc.sync.dma_start(wq_sb[:, g, :], wqt[g * 128:(g + 1) * 128, :])
        xq_sb = x_pool.tile([128, CI, NPC], F32R, tag="x")
        for g in range(CI):
            nc.sync.dma_start(xq_sb[:, g, :], xq[g * 128:(g + 1) * 128, :])

        for ch in range(NCHUNK):
            q_ps = ps_pool.tile([128, C], F32, tag="kv")
            cs = slice(ch * CHUNK, (ch + 1) * CHUNK)
            for half in range(2):
                hs = slice(half * 512, (half + 1) * 512)
                for g in range(CI):
                    nc.tensor.matmul(
                        q_ps[:, hs], xq_sb[:, g, cs], wq_sb[:, g, hs],
                        start=(g == 0), stop=(g == CI - 1),
                    )
            nc.scalar.copy(q_sb[:, ch, :], q_ps[:])

        # ---- online attention over l, AV delayed one chunk-iteration ----
        # The exp (ACT) for (l, ch) is consumed by the AV update one
        # iteration later, so the DVE never stalls on the ACT round-trip.
        prev = None  # (v_ps, ch, l) whose e is already requested

        def flush_prev():
            v_prev, chp, lp = prev
            m_sb = p_pool.tile([128, H, D], F32, tag="m", name=f"m_{chp}_{lp}")
            nc.vector.tensor_mul(
                m_sb[:],
                v_prev[:].rearrange("p (h d) -> p h d", d=D),
                e_all[:, chp, lp, :].unsqueeze(2).broadcast_to((128, H, D)),
            )
            nc.vector.tensor_add(num_sb[:, chp], num_sb[:, chp], m_sb[:])
        for l in range(L):
            wk_sb = wk_pool.tile([128, CI, C], F32R, tag="w")
            for g in range(CI):
                nc.sync.dma_start(wk_sb[:, g, :], wkt[l, g * 128:(g + 1) * 128, :])
            x_sb = x_pool.tile([128, CI, NPC], F32R, tag="x")
            for g in range(CI):
                nc.sync.dma_start(x_sb[:, g, :], xt[l, g * 128:(g + 1) * 128, :])

            for ch in range(NCHUNK):
                cs = slice(ch * CHUNK, (ch + 1) * CHUNK)
                k_ps = ps_pool.tile([128, C], F32, tag="kv")
                v_ps = ps_pool.tile([128, C], F32, tag="kv")
                for half in range(2):
                    hs = slice(half * 512, (half + 1) * 512)
                    for g in range(CI):
                        nc.tensor.matmul(
                            k_ps[:, hs], x_sb[:, g, cs], wk_sb[:, g, hs],
                            start=(g == 0), stop=(g == CI - 1),
                        )
                for half in range(2):
                    hs = slice(half * 512, (half + 1) * 512)
                    for g in range(CI):
                        nc.tensor.matmul(
                            v_ps[:, hs], x_sb[:, g, cs], wv_sb[:, g, hs],
                            start=(g == 0), stop=(g == CI - 1),
                        )

                # scores: s[n, h] = sum_d q[n, h, d] * k[n, h, d]
                p_sb = p_pool.tile([128, H, D], F32, tag="p")
                nc.vector.tensor_mul(
                    p_sb[:],
                    q_sb[:, ch, :].rearrange("p (h d) -> p h d", d=D),
                    k_ps[:].rearrange("p (h d) -> p h d", d=D),
                )
                s_sb = small_pool.tile([128, H], F32, tag="s")
                nc.vector.tensor_reduce(
                    s_sb[:], p_sb[:], axis=mybir.AxisListType.X,
                    op=mybir.AluOpType.add,
                )
                nc.scalar.activation(
                    e_all[:, ch, l, :], s_sb[:],
                    mybir.ActivationFunctionType.Exp,
                )
                if prev is not None:
                    flush_prev()
                prev = (v_ps, ch, l)
        flush_prev()

        # ---- normalize + output projection ----
        wo_sb = wk_pool.tile([128, CI, C], F32R, tag="w")
        for g in range(CI):
            nc.sync.dma_start(wo_sb[:, g, :], wot[g * 128:(g + 1) * 128, :])

        for ch in range(NCHUNK):
            den = small_pool.tile([128, H], F32, tag="den")
            nc.vector.tensor_reduce(
                den[:],
                e_all[:, ch].transpose([0, 2, 1]),
                axis=mybir.AxisListType.X, op=mybir.AluOpType.add,
            )
            rden = small_pool.tile([128, H], F32, tag="rd")
            nc.vector.reciprocal(rden[:], den[:])
            att_sb = o_pool.tile([128, H, D], F32, tag="att")
            nc.vector.tensor_mul(
                att_sb[:], num_sb[:, ch],
                rden[:].unsqueeze(2).broadcast_to((128, H, D)),
            )
            att2 = att_sb[:].rearrange("p h d -> p (h d)")
            t_ps = ps_pool.tile([128, C], F32, tag="kv")
            for g in range(CI):
                nc.tensor.transpose(
                    t_ps[:, g * 128:(g + 1) * 128],
                    att2[:, g * 128:(g + 1) * 128],
                    id_sb[:],
                )
            attT = o_pool.tile([128, CI, 128], F32R, tag="attT")
            nc.vector.tensor_copy(
                attT[:].rearrange("p g n -> p (g n)"), t_ps[:]
            )
            o_ps = ps_pool.tile([128, C], F32, tag="kv")
            for half in range(2):
                hs = slice(half * 512, (half + 1) * 512)
                for g in range(CI):
                    nc.tensor.matmul(
                        o_ps[:, hs], attT[:, g, :], wo_sb[:, g, hs],
                        start=(g == 0), stop=(g == CI - 1),
                    )
            out_sb = o_pool.tile([128, C], F32, tag="out")
            nc.scalar.copy(out_sb[:], o_ps[:])
            nc.sync.dma_start(out[ch * CHUNK:(ch + 1) * CHUNK, :], out_sb[:])


def _build_bass():
    nc = bacc.Bacc("TRN2", target_bir_lowering=False, debug=False,
                   num_devices=N_CORES)
    aps = {
        "xt": nc.dram_tensor("xt", (L, C, NPC), F32R, kind="ExternalInput").ap(),
        "xq": nc.dram_tensor("xq", (C, NPC), F32R, kind="ExternalInput").ap(),
        "wkt": nc.dram_tensor("wkt", (L, C, C), F32R, kind="ExternalInput").ap(),
        "wvt": nc.dram_tensor("wvt", (C, C), F32R, kind="ExternalInput").ap(),
        "wqt": nc.dram_tensor("wqt", (C, C), F32R, kind="ExternalInput").ap(),
        "wot": nc.dram_tensor("wot", (C, C), F32R, kind="ExternalInput").ap(),
        "ident": nc.dram_tensor("ident", (128, 128), F32, kind="ExternalInput").ap(),
        "out": nc.dram_tensor("out", (NPC, C), F32, kind="ExternalOutput").ap(),
    }
    with tile.TileContext(nc) as tc:
        _emit(tc, aps)
    nc.compile()
    return nc


def _rope_tables():
    inv_freq = 1.0 / (ROPE_BASE ** (np.arange(0, D, 2, dtype=np.float32) / D))
    freqs = np.arange(L, dtype=np.float32)[:, None] * inv_freq[None, :]
    emb = np.concatenate([freqs, freqs], axis=-1)          # (L, D)
    return np.cos(emb).astype(np.float32), np.sin(emb).astype(np.float32)


def _rope_weight(w, cos_l, sin_l):
    """R_l @ W for a (C, C) projection weight (rows indexed by h*D+d)."""
    w3 = w.reshape(H, D, C)
    rot = np.concatenate([-w3[:, D // 2:, :], w3[:, :D // 2, :]], axis=1)
    return (cos_l[None, :, None] * w3 + sin_l[None, :, None] * rot).reshape(C, C)


def _host_prep(layer_outputs, Wq, Wk, Wv, Wo):
    cos, sin = _rope_tables()
    wkt = np.empty((L, C, C), dtype=np.float32)
    for l in range(L):
        wkt[l] = np.ascontiguousarray(_rope_weight(Wk, cos[l], sin[l]).T)
    wq12 = _rope_weight(Wq, cos[L - 1], sin[L - 1]) / np.float32(np.sqrt(D))
    shared = {
        "wkt": wkt,
        "wvt": np.ascontiguousarray(Wv.T),
        "wqt": np.ascontiguousarray(wq12.T.astype(np.float32)),
        "wot": np.ascontiguousarray(Wo.T),
        "ident": np.eye(128, dtype=np.float32),
    }
    in_maps = []
    for c in range(N_CORES):
        b = c // (T // NPC) if NPC <= T else c
        # rows n = b*T + t, core c covers n in [c*NPC, (c+1)*NPC)
        n0 = c * NPC
        b = n0 // T
        t0 = n0 % T
        sl = layer_outputs[:, b, t0:t0 + NPC, :]          # (L, NPC, C)
        xt = np.ascontiguousarray(sl.transpose(0, 2, 1))  # (L, C, NPC)
        in_maps.append({
            "xt": xt,
            "xq": np.ascontiguousarray(xt[L - 1]),
            **shared,
        })
    return in_maps


def _get_nc():
    global _CACHED_NC
    if _CACHED_NC is None:
        _CACHED_NC = _build_bass()
    return _CACHED_NC


def kernel(layer_outputs, Wq, Wk, Wv, Wo):
    layer_outputs = np.asarray(layer_outputs, dtype=np.float32)
    Wq = np.asarray(Wq, dtype=np.float32)
    Wk = np.asarray(Wk, dtype=np.float32)
    Wv = np.asarray(Wv, dtype=np.float32)
    Wo = np.asarray(Wo, dtype=np.float32)

    nc = _get_nc()
    in_maps = _host_prep(layer_outputs, Wq, Wk, Wv, Wo)
    res = run_bass_kernel_spmd(nc, in_maps, core_ids=list(range(N_CORES)))
    full = np.concatenate([r["out"] for r in res.results], axis=0)  # (B*T, C)
    return full.reshape(B, T, C)


if __name__ == "__main__":
    nc = _build_bass()
    print("build OK:",
          sum(len(f.blocks[0].instructions) if f.blocks else 0
              for f in nc.m.functions) if hasattr(nc.m.functions[0], 'blocks')
          else "n/a")
